# revision 32
# baseline (speedup 1.0000x reference)
"""BertAttention Trainium2 kernel — 8-core SPMD, v5.

Sharding: each core owns 2 heads (128 of the 1024 feature dims); output
tokens are interleaved per i-block: for each of the 8 512-token i-blocks,
core c owns tokens [ib_start + 64*c, +64).

v5 design:
  - ONE DMA instruction per 512-token x quarter (host pre-tiles x as
    [B, 4, 128, CCH, 512]) — DMA_DIRECT2D issue costs ~600ns on the
    sequencer, so the v4 scheme of 8 chunk-DMAs per quarter was
    issue-rate-bound, starving the PE for the first ~80us.
  - Weights/projection/exchange transfers likewise single-DMA via
    host re-tiling or AP dim reordering.
  - Minimal prologue; all other QKV/v units are in-loop PE fillers.
  - Pair-level AllToAll (4 x 256KB) triggered late (x-stream drained,
    CC barrier passed); out-proj + residual + LayerNorm run mid-loop
    per pair; only the last pair drains after the loop.
  - LayerNorm rstd = exp(-0.5*ln(var+eps)): Exp and Ln share one
    activation table -> zero table thrash on the scalar engine.
"""

import os
import sys

for _p in ("/opt/trn_rl_repo", "/root/.axon_site/_ro/trn_rl_repo"):
    if os.path.isdir(_p) and _p not in sys.path:
        sys.path.append(_p)

import ml_dtypes
import numpy as np

try:
    import antenv.axon_hooks  # noqa: F401
except Exception:
    import types as _types
    try:
        import antenv as _antenv
        _m = _types.ModuleType("antenv.axon_hooks")
        _m._hook = None
        _m.set_axon_ntff_profile_hook = lambda h, _m=_m: setattr(_m, "_hook", h)
        _m.get_axon_ntff_profile_hook = lambda _m=_m: _m._hook
        sys.modules["antenv.axon_hooks"] = _m
        _antenv.axon_hooks = _m
    except Exception:
        pass

import concourse.bass as bass  # noqa: F401
import concourse.tile as tile
from concourse import bacc, mybir
from concourse.bass_utils import run_bass_kernel_spmd

F32 = mybir.dt.float32
BF16 = mybir.dt.bfloat16
BF16_NP = ml_dtypes.bfloat16

NCORES = 8
H = 16   # heads total
DH = 64  # head dim
LN_EPS = 1e-12


def build_bert_kernel(S=2048, B=2, D=1024):
    P = 128
    NTOK = S * B              # 4096 batch-major tokens
    TPC = NTOK // NCORES      # 512 output tokens per core
    CCH = D // P              # 8 contraction chunks
    HPC = H // NCORES         # 2 heads per core
    DL = HPC * DH             # 128 local feature dims
    NJ = S // P               # 16 key chunks per batch
    NI = S // 512             # 4 query blocks per batch
    NSLOT = B * NI * NJ       # 128 score chunk-slots
    NIB = B * NI              # 8 i-blocks (512 tokens each)
    NVT = NTOK // P           # 32 v token tiles

    nc = bacc.Bacc("TRN2", target_bir_lowering=False, debug=False,
                   num_devices=NCORES)

    def din(name, shape, dt=F32):
        return nc.dram_tensor(name, list(shape), dt, kind="ExternalInput").ap()

    # x host-tiled [B, 4, 128, CCH, 512]: one contiguous 1MB DMA per
    # (proj, batch, 512-token quarter)
    xqT = din("xqT", (B, 4, P, CCH, 512), BF16)
    xkT = din("xkT", (B, 4, P, CCH, 512), BF16)
    xvT = din("xvT", (B, 4, P, CCH, 512), BF16)
    # weights host-tiled [128, CCH, out]: one DMA each
    wqT = din("wqT", (P, CCH, DL), BF16)
    wkT = din("wkT", (P, CCH, DL), BF16)
    wvT = din("wvT", (P, CCH, DL), BF16)
    woT = din("woT", (P, CCH, D), BF16)
    bq = din("bq", (DL, 1))
    bk = din("bk", (DL, 1))
    bv = din("bv", (1, DL))
    bo = din("bo", (1, D), BF16)
    lnw = din("lnw", (1, D), BF16)
    lnb = din("lnb", (1, D), BF16)
    resid = din("resid", (TPC, D))
    out = nc.dram_tensor("out", [TPC, D], BF16, kind="ExternalOutput").ap()

    # per-PAIR exchange (i-blocks 2m, 2m+1): piece p of a2a[m] = this
    # core's ctx^T (128 dims) for dest core p's 2x64 tokens.
    a2a = [nc.dram_tensor(f"a2a{m}", [NCORES, P, P], BF16).ap()
           for m in range(NIB // 2)]
    ag = [nc.dram_tensor(f"ag{m}", [NCORES, P, P], BF16).ap()
          for m in range(NIB // 2)]
    # DRAM bounce buffers for the softmax-reciprocal partition broadcast
    # (partition_broadcast is a gpsimd op and the gpsimd queue blocks on
    # in-flight collectives, so broadcast via two sync-queue DMAs instead)
    rcp_d = [nc.dram_tensor(f"rcpd{i}", [1, 512], F32).ap()
             for i in range(4)]
    GRP = [list(range(NCORES))]

    with tile.TileContext(nc) as tc:
        with (
            tc.tile_pool(name="persist", bufs=1) as persist,
            tc.tile_pool(name="small", bufs=1) as small,
            tc.tile_pool(name="xk_p", bufs=1) as xk_pool,
            tc.tile_pool(name="xq_p", bufs=1) as xq_pool,
            tc.tile_pool(name="xv_p", bufs=1) as xv_pool,
            tc.tile_pool(name="work", bufs=1) as work,
            tc.tile_pool(name="ps_sc", bufs=1, space="PSUM") as ps_sc,
            tc.tile_pool(name="ps_cps", bufs=1, space="PSUM") as ps_cps,
        ):
            SCB = 3  # scores/filler PSUM ring depth (3 x 2 banks)

            def sc_tile():
                return ps_sc.tile([P, 2 * 512], F32, name="sc",
                                  tag="sc", bufs=SCB)

            # ---- SBUF tiles ----
            wqT_sb = persist.tile([P, CCH, DL], BF16)
            wkT_sb = persist.tile([P, CCH, DL], BF16)
            wvT_sb = persist.tile([P, CCH, DL], BF16)
            woT_sb = persist.tile([P, CCH, D], BF16)

            bq_sb = small.tile([P, 1], F32)
            bk_sb = small.tile([P, 1], F32)
            bv_bc = small.tile([P, DL], F32)
            bo_bc = small.tile([P, D], BF16)
            lnw_bc = small.tile([P, D], BF16)
            lnb_bc = small.tile([P, D], BF16)
            eps_sb = small.tile([P, 1], F32)
            nc.vector.memset(eps_sb, LN_EPS)

            qT_sb = persist.tile([P, NTOK], BF16)   # [dloc, tok]
            kT_sb = persist.tile([P, NTOK], BF16)
            v_sb = persist.tile([P, NVT, HPC * (DH + 1)], BF16)
            nc.vector.memset(v_sb[:, :, DH:DH + 1], 1.0)
            nc.vector.memset(v_sb[:, :, 2 * DH + 1:2 * DH + 2], 1.0)
            ERD = 6
            e_ring = persist.tile([P, ERD, 2 * 512], BF16)  # exp ring
            resid_sb = {}
            ctxPair = {}
            y_sb = {}

            # ---- x tiles: [128, CCH, 512] per (proj, batch, quarter) ----
            xt = {}
            xd = {0: (xqT, xq_pool, "xq"), 1: (xkT, xk_pool, "xk"),
                  2: (xvT, xv_pool, "xv")}
            for ti in (0, 1, 2):
                for b in range(B):
                    for n in range(4):
                        _, pool, tag = xd[ti]
                        xt[(ti, b, n)] = pool.tile(
                            [P, CCH, 512], BF16, name=f"x{tag}_{b}_{n}",
                            tag=tag, bufs=4)

            def xdma(q, ti, b, n):
                q.dma_start(out=xt[(ti, b, n)], in_=xd[ti][0][b, n])

            SY, GP, SC = nc.sync, nc.gpsimd, nc.scalar

            # b0 stream split across sync+gpsimd so the prologue's kT
            # (all 4 xk quarters) lands in ~7us instead of ~13us
            # prologue-critical first: kT quarters 0-3 split across both
            # queues, then xq q0 / xv q0, then the rest by deadline
            SY.dma_start(out=bq_sb, in_=bq)
            SY.dma_start(out=bk_sb, in_=bk)
            SY.dma_start(out=wkT_sb, in_=wkT)
            xdma(SY, 1, 0, 0)
            SY.dma_start(out=wqT_sb, in_=wqT)
            xdma(SY, 1, 0, 2)
            xdma(SY, 0, 0, 0)
            xdma(SY, 0, 0, 1)
            xdma(SY, 0, 0, 2)
            xdma(SY, 0, 0, 3)

            GP.dma_start(out=wvT_sb, in_=wvT)
            xdma(GP, 1, 0, 1)
            xdma(GP, 1, 0, 3)
            xdma(GP, 2, 0, 0)
            xdma(GP, 2, 0, 1)
            xdma(GP, 2, 0, 2)
            xdma(GP, 2, 0, 3)

            # scalar queue: small consts pre-loop (bv needed by the
            # prologue v unit), woT/resid via fillers
            SC.dma_start(out=bv_bc, in_=bv.to_broadcast((P, DL)))

            def load_consts():
                SC.dma_start(out=bo_bc, in_=bo.to_broadcast((P, D)))
                SC.dma_start(out=lnw_bc, in_=lnw.to_broadcast((P, D)))
                SC.dma_start(out=lnb_bc, in_=lnb.to_broadcast((P, D)))

            def load_woT():
                SC.dma_start(out=woT_sb, in_=woT)

            def load_resid(m):
                rt = work.tile([P, D], F32, tag="resid", bufs=2,
                               name=f"resid{m}")
                resid_sb[m] = rt
                SC.dma_start(out=rt, in_=resid[m * P:(m + 1) * P, :])

            # ================= unit builders =================
            def qk_unit(ti, b, n):
                w_sb, b_sb, o_sb = ((wqT_sb, bq_sb, qT_sb),
                                    (wkT_sb, bk_sb, kT_sb))[ti]
                ps = sc_tile()[:, 0:512]
                xtile = xt[(ti, b, n)]
                for c in range(CCH):
                    nc.tensor.matmul(ps, w_sb[:, c, :], xtile[:, c, :],
                                     start=(c == 0), stop=(c == CCH - 1))
                tok0 = b * S + n * 512
                nc.vector.tensor_scalar_add(o_sb[:, tok0:tok0 + 512], ps, b_sb)

            def v_unit(b, nt):
                # v projection for 4 token tiles (512 tokens)
                ps = sc_tile()[:, 0:512]
                xtile = xt[(2, b, nt)]
                for k in range(4):
                    for c in range(CCH):
                        nc.tensor.matmul(
                            ps[:, k * P:(k + 1) * P],
                            xtile[:, c, k * P:(k + 1) * P],
                            wvT_sb[:, c, :],
                            start=(c == 0), stop=(c == CCH - 1))
                for k in range(4):
                    it = b * (S // P) + nt * 4 + k
                    dst = v_sb[:, it, :].rearrange(
                        "p (h x) -> p h x", h=HPC)[:, :, 0:DH]
                    src = ps[:, k * P:(k + 1) * P].rearrange(
                        "p (h x) -> p h x", h=HPC)
                    bvr = bv_bc.rearrange("p (h x) -> p h x", h=HPC)
                    nc.vector.tensor_add(dst, src, bvr)

            def trig_unit(m):
                nc.gpsimd.collective_compute(
                    "AllToAll", mybir.AluOpType.bypass,
                    replica_groups=GRP,
                    ins=[a2a[m].opt()], outs=[ag[m].opt()])

            def recv_unit(m):
                # on gpsimd right after trig_unit(m): the gpsimd queue is
                # already blocked on that collective, so waiting for ag[m]
                # there couples nothing else
                ct = work.tile([P, CCH, P], BF16, tag="ctxP", bufs=2,
                               name=f"ctxP{m}")
                ctxPair[m] = ct
                nc.gpsimd.dma_start(out=ct,
                                    in_=ag[m].rearrange("c d t -> d c t"))

            def op_unit(m, nh):
                # output projection for token pair-block m (128 tokens),
                # output dims [nh*512, (nh+1)*512)
                ps = sc_tile()[:, 0:512]
                if nh == 0:
                    yt = work.tile([P, D], F32, tag="y", bufs=2,
                                   name=f"y{m}")
                    y_sb[m] = yt
                for c in range(CCH):
                    nc.tensor.matmul(ps, ctxPair[m][:, c, :],
                                     woT_sb[:, c, nh * 512:(nh + 1) * 512],
                                     start=(c == 0), stop=(c == CCH - 1))
                sl = slice(nh * 512, (nh + 1) * 512)
                y = y_sb[m]
                nc.vector.tensor_add(y[:, sl], ps, bo_bc[:, sl])
                nc.vector.tensor_add(y[:, sl], y[:, sl], resid_sb[m][:, sl])

            def ln_unit(m):
                y = y_sb[m]
                y3 = y.rearrange("p (g d) -> p g d", g=2)
                stats = work.tile([P, 2, 6], F32, tag="stats", bufs=2)
                for g in range(2):
                    nc.vector.bn_stats(out=stats[:, g, :], in_=y3[:, g, :])
                mv = work.tile([P, 2], F32, tag="mv", bufs=2)
                nc.vector.bn_aggr(out=mv, in_=stats)
                # rstd = exp(-0.5*ln(var+eps)): Exp+Ln share one table
                lnv = work.tile([P, 1], F32, tag="lnv", bufs=2)
                nc.scalar.activation(lnv, mv[:, 1:2],
                                     mybir.ActivationFunctionType.Ln,
                                     bias=eps_sb)
                rstd = work.tile([P, 1], F32, tag="rstd", bufs=2)
                nc.scalar.activation(rstd, lnv,
                                     mybir.ActivationFunctionType.Exp,
                                     scale=-0.5)
                nc.vector.tensor_scalar(
                    out=y, in0=y, scalar1=mv[:, 0:1], scalar2=rstd,
                    op0=mybir.AluOpType.subtract, op1=mybir.AluOpType.mult)
                of = work.tile([P, D], BF16, tag="of", bufs=2)
                nc.vector.tensor_mul(of, y, lnw_bc)
                nc.vector.tensor_add(of, of, lnb_bc)
                nc.sync.dma_start(out=out[m * P:(m + 1) * P, :], in_=of)

            # ---- filler placement: slot -> list of unit closures ----
            fillers = {}

            def add_filler(s, fn):
                fillers.setdefault(s, []).append(fn)

            sched = [
                (2, load_consts),
                (3, lambda: v_unit(0, 1)),
                (6, load_woT),
                (7, lambda: v_unit(0, 2)),
                (10, lambda: load_resid(0)),
                (11, lambda: v_unit(0, 3)),
                (13, lambda: qk_unit(0, 0, 1)),
                (15, lambda: qk_unit(0, 0, 2)),
                (17, lambda: qk_unit(0, 0, 3)),
                # b1 x loads (one DMA each); emitted as soon as the x
                # ring slot they reuse frees so the whole x-stream lands
                # before the first collective fires
                (2, lambda: xdma(GP, 1, 1, 0)),
                (3, lambda: xdma(GP, 1, 1, 1)),
                (4, lambda: xdma(GP, 1, 1, 2)),
                (5, lambda: xdma(GP, 1, 1, 3)),
                (6, lambda: xdma(GP, 0, 1, 0)),
                (7, lambda: xdma(GP, 0, 1, 1)),
                (8, lambda: xdma(GP, 0, 1, 2)),
                (9, lambda: xdma(GP, 0, 1, 3)),
                (10, lambda: xdma(SY, 2, 1, 0)),
                (11, lambda: xdma(SY, 2, 1, 1)),
                (12, lambda: xdma(SY, 2, 1, 2)),
                (13, lambda: xdma(SY, 2, 1, 3)),
                (30, lambda: qk_unit(1, 1, 0)),
                (33, lambda: qk_unit(1, 1, 1)),
                (36, lambda: qk_unit(1, 1, 2)),
                (39, lambda: qk_unit(1, 1, 3)),
                (42, lambda: qk_unit(0, 1, 0)),
                (45, lambda: qk_unit(0, 1, 1)),
                (48, lambda: qk_unit(0, 1, 2)),
                (51, lambda: v_unit(1, 0)),
                (52, lambda: load_resid(1)),
                (54, lambda: v_unit(1, 1)),
                (57, lambda: v_unit(1, 2)),
                (60, lambda: v_unit(1, 3)),
                (64, lambda: qk_unit(0, 1, 3)),
                (62, lambda: trig_unit(0)),
                (63, lambda: recv_unit(0)),
                (66, lambda: trig_unit(1)),
                (67, lambda: recv_unit(1)),
                (82, lambda: op_unit(0, 0)),
                (84, lambda: op_unit(0, 1)),
                (86, lambda: ln_unit(0)),
                (94, lambda: op_unit(1, 0)),
                (96, lambda: op_unit(1, 1)),
                (97, lambda: load_resid(2)),
                (98, lambda: ln_unit(1)),
                (101, lambda: trig_unit(2)),
                (102, lambda: recv_unit(2)),
                (118, lambda: op_unit(2, 0)),
                (120, lambda: op_unit(2, 1)),
                (122, lambda: ln_unit(2)),
                (123, lambda: load_resid(3)),
            ]
            for s, fn in sched:
                add_filler(s, fn)

            # ================= main slot loop =================
            cps = [None, None]
            ctxo = {}

            def scores_step(s):
                b, r = divmod(s, NI * NJ)
                i, j = divmod(r, NJ)
                sc = sc_tile()
                jc0 = b * S + j * P
                ic0 = b * S + i * 512
                for h in range(HPC):
                    nc.tensor.matmul(
                        sc[:, h * 512:(h + 1) * 512],
                        kT_sb[h * DH:(h + 1) * DH, jc0:jc0 + P],
                        qT_sb[h * DH:(h + 1) * DH, ic0:ic0 + 512])
                nc.scalar.activation(e_ring[:, s % ERD, :], sc,
                                     mybir.ActivationFunctionType.Exp)

            def ctx_step(c, h):
                b, r = divmod(c, NI * NJ)
                i, j = divmod(r, NJ)
                vt = b * NJ + j
                if j == 0:
                    cps[h] = ps_cps.tile([DH + 1, 512], F32, name=f"cps{h}",
                                         tag=f"cps{h}", bufs=1)
                nc.tensor.matmul(
                    cps[h],
                    v_sb[:, vt, h * (DH + 1):(h + 1) * (DH + 1)],
                    e_ring[:, c % ERD, h * 512:(h + 1) * 512],
                    start=(j == 0), stop=(j == NJ - 1))
                if j != NJ - 1:
                    return
                # ---- softmax normalize + stage for exchange ----
                n = b * NI + i
                if h == 0:
                    ctxo[n] = work.tile([P, 512], BF16, tag="ctxo",
                                        bufs=2, name=f"ctxo{n}")
                co = ctxo[n]
                cs = work.tile([DH, 512], F32, tag="cs", bufs=2)
                nc.vector.tensor_copy(cs, cps[h][0:DH, :])
                ssum = work.tile([1, 512], F32, tag="ssum", bufs=2)
                nc.vector.tensor_copy(ssum, cps[h][DH:DH + 1, :])
                rcp = work.tile([1, 512], F32, tag="rcp", bufs=2)
                nc.vector.reciprocal_approx_fast(rcp, ssum)
                rd = rcp_d[(2 * n + h) % 4]
                nc.sync.dma_start(out=rd, in_=rcp)
                rbc = work.tile([DH, 512], F32, tag="rbc", bufs=2)
                nc.sync.dma_start(out=rbc, in_=rd.to_broadcast((DH, 512)))
                nc.vector.tensor_mul(co[h * DH:(h + 1) * DH, :],
                                     cs, rbc)
                if h == HPC - 1:
                    # stage this i-block's half of the pair exchange:
                    # one DMA, co[d, p*64+t] -> a2a[m][p, d, 64h+t]
                    m, half = divmod(n, 2)
                    dst = a2a[m].rearrange(
                        "p d (g t) -> d p g t", g=2)[:, :, half, :]
                    nc.sync.dma_start(out=dst, in_=co)

            # prologue: full kT b0, qT b0 i0, v tiles 0-3
            for n in range(4):
                qk_unit(1, 0, n)
            qk_unit(0, 0, 0)
            v_unit(0, 0)

            for s in range(NSLOT + 3):
                if s < NSLOT:
                    scores_step(s)
                if 0 <= s - 2 < NSLOT:
                    ctx_step(s - 2, 0)
                if 0 <= s - 3 < NSLOT:
                    ctx_step(s - 3, 1)
                for fn in fillers.get(s, ()):
                    fn()

            # epilogue: last pair's exchange + projection + LayerNorm
            trig_unit(3)
            recv_unit(3)
            op_unit(3, 0)
            op_unit(3, 1)
            ln_unit(3)

    nc.compile()
    return nc


_NC_CACHE = {}


def _get_nc(S=2048, B=2, D=1024):
    key = (S, B, D)
    if key not in _NC_CACHE:
        _NC_CACHE[key] = build_bert_kernel(S, B, D)
    return _NC_CACHE[key]


def make_in_maps(query_tensor, key_tensor, value_tensor, Wq, bq, Wk, bk,
                 Wv, bv, Wo, bo, ln_w, ln_b):
    S, B, D = query_tensor.shape
    NTOK = S * B
    P = 128
    CCH = D // P
    DL = (H // NCORES) * DH
    NIB = NTOK // 512

    def bm(x):  # (S, B, D) -> batch-major (B*S, D) float32
        return np.ascontiguousarray(
            np.asarray(x, np.float32).transpose(1, 0, 2).reshape(NTOK, D))

    def x_tiled(x):  # -> [B, 4, 128, CCH, 512] bf16
        a = bm(x).T.astype(BF16_NP)                 # (D, B*S)
        a = a.reshape(CCH, P, B, 4, 512)
        return np.ascontiguousarray(a.transpose(2, 3, 1, 0, 4))

    def w_tiled(w):  # (DL_out rows of W) -> [128, CCH, DL_out] bf16
        a = np.asarray(w, np.float32).T.astype(BF16_NP)   # (D, DL)
        a = a.reshape(CCH, P, -1)
        return np.ascontiguousarray(a.transpose(1, 0, 2))

    xq = bm(query_tensor)
    xqT = x_tiled(query_tensor)
    xkT = x_tiled(key_tensor)
    xvT = x_tiled(value_tensor)
    woT = w_tiled(Wo)          # [128, CCH, D]
    f32 = lambda a: np.ascontiguousarray(np.asarray(a, np.float32))
    in_maps = []
    for c in range(NCORES):
        sl = slice(c * DL, (c + 1) * DL)
        rs = np.concatenate(
            [xq[n * 512 + c * 64:n * 512 + (c + 1) * 64]
             for n in range(NIB)], axis=0)
        in_maps.append({
            "xqT": xqT, "xkT": xkT, "xvT": xvT,
            "wqT": w_tiled(Wq[sl]), "wkT": w_tiled(Wk[sl]),
            "wvT": w_tiled(Wv[sl]), "woT": woT,
            "bq": f32(bq[sl]).reshape(DL, 1),
            "bk": f32(bk[sl]).reshape(DL, 1),
            "bv": f32(bv[sl]).reshape(1, DL),
            "bo": np.ascontiguousarray(
                np.asarray(bo, np.float32).astype(BF16_NP)).reshape(1, D),
            "lnw": np.ascontiguousarray(
                np.asarray(ln_w, np.float32).astype(BF16_NP)).reshape(1, D),
            "lnb": np.ascontiguousarray(
                np.asarray(ln_b, np.float32).astype(BF16_NP)).reshape(1, D),
            "resid": np.ascontiguousarray(rs),
        })
    return in_maps


def assemble_output(results, S, B, D):
    NTOK = S * B
    NIB = NTOK // 512
    full = np.empty((NTOK, D), np.float32)
    for c, r in enumerate(results):
        o = np.asarray(r["out"], np.float32)  # (512, D) bf16 -> f32
        for n in range(NIB):
            full[n * 512 + c * 64:n * 512 + (c + 1) * 64] = \
                o[n * 64:(n + 1) * 64]
    return np.ascontiguousarray(
        full.reshape(B, S, D).transpose(1, 0, 2))


def kernel(**inputs):
    S, B, D = inputs["query_tensor"].shape
    nc = _get_nc(S, B, D)
    in_maps = make_in_maps(**inputs)
    res = run_bass_kernel_spmd(nc, in_maps, list(range(NCORES)))
    return assemble_output(res.results, S, B, D)
